# revision 51
# baseline (speedup 1.0000x reference)
"""
Trainium2 Bass kernel for nn_BMM_S8T_S8N_S8T:
  y[b,m,n] = sat_i8(round(alpha * sum_k a[b,m,k] * b[b,n,k]))
with a,b int8 [128, 1024, 128], alpha scalar.

Strategy (8 NeuronCores, batch-parallel, 16 batches/core):
 - Host: pre-transpose a -> [BPC, K, M], b -> [BPC, K, N] so SBUF tiles land
   directly in [contraction-partition, free] layout. No on-chip transposes.
 - Input DMA on SWDGE (gpsimd) casts int8 -> bf16 in the DMA datapath; the
   engines never touch input conversion. bf16 holds int8 exactly; products
   (<= 2^14) and fp32 accumulations (|acc| <= 2^21) are bit-exact.
 - First NBF batches ship host-prepared bf16 (HWDGE scalar ring) so drains
   start ~user 12us; PE pre-warms on dummy matmuls so HAM is at 2.4 GHz when
   real matmuls land.
 - Matmuls: per batch, 8 stationary A-tiles [128k, 128m] x moving B [128k,
   512n] pairs into [128, 1024] fp32 PSUM tiles (2 banks, 4 in flight).
 - Epilogue (the critical path, ~75us): one op per PSUM tile: int8 out =
   rne_sat(alpha*acc), strict ACT/DVE interleave with 3 extra ACT tiles
   (67:61 matches 1122ns vs 1215ns measured per-op costs). 1024-el ops are
   optimal: larger ops cross a PSUM bank-pair boundary (+~540 cycles).
 - Stores: half-batch 512 KiB DMAs on the sync ring only (scalar would steal
   ACT sequencer time; SWDGE descriptor-gen would thrash SBUF ports).
"""

import sys

sys.path.insert(0, "/opt/trn_rl_repo")

import numpy as np

N_CORES = 8
B, M, N, K = 128, 1024, 1024, 128
BPC = B // N_CORES  # batches per core
MT = M // 128
HALF = BPC // 2
NBF = 6  # leading batches shipped as host-prepared bf16 (prologue fast path)

_cache = {}


def _build(alpha: float):
    import concourse.bacc as bacc
    import concourse.tile as tile
    import concourse.mybir as mybir

    nc = bacc.Bacc("TRN2", target_bir_lowering=False, debug=False)

    a_t = nc.dram_tensor("a_t", [BPC, K, M], mybir.dt.int8, kind="ExternalInput")
    b_t = nc.dram_tensor("b_t", [BPC, K, N], mybir.dt.int8, kind="ExternalInput")
    # host-prepared bf16 copies of the first NBF batches (prologue fast path:
    # HWDGE loads them directly, engines never do input conversion)
    a_bf = nc.dram_tensor("a_bf", [NBF, K, M], mybir.dt.bfloat16, kind="ExternalInput")
    b_bf = nc.dram_tensor("b_bf", [NBF, K, N], mybir.dt.bfloat16, kind="ExternalInput")
    y = nc.dram_tensor("y", [BPC, M, N], mybir.dt.int8, kind="ExternalOutput")

    bf16 = mybir.dt.bfloat16
    f32 = mybir.dt.float32
    i8 = mybir.dt.int8

    a_v = a_t.rearrange("b k m -> k b m")  # [128, BPC, 1024]
    b_v = b_t.rearrange("b k n -> k b n")

    with tile.TileContext(nc) as tc:
        with (
            tc.tile_pool(name="inp", bufs=1) as ipool,
            tc.tile_pool(name="outp", bufs=5) as opool,
            tc.tile_pool(name="ps", bufs=4, space="PSUM") as pspool,
        ):
            # input tiles: all 16 batches resident as bf16 (64 KB/partition).
            # One tile per DMA writer: a tile written by multiple DMAs gets a
            # coarse "all writers done" readiness sem that stalls consumers
            # needing only the first writer.
            a_h0 = ipool.tile([128, M], bf16, tag="ah0")
            b_h0 = ipool.tile([128, N], bf16, tag="bh0")
            a_h1 = ipool.tile([128, M], bf16, tag="ah1")
            b_h1 = ipool.tile([128, N], bf16, tag="bh1")
            a_h23 = ipool.tile([128, 2, M], bf16, tag="ah23")
            b_h23 = ipool.tile([128, 2, N], bf16, tag="bh23")
            a_h45 = ipool.tile([128, 2, M], bf16, tag="ah45")
            b_h45 = ipool.tile([128, 2, N], bf16, tag="bh45")
            a_mid = ipool.tile([128, 8 - NBF, M], bf16, tag="amid")
            b_mid = ipool.tile([128, 8 - NBF, N], bf16, tag="bmid")
            a_hi = ipool.tile([128, HALF, M], bf16, tag="ahi")
            b_hi = ipool.tile([128, HALF, N], bf16, tag="bhi")

            def a_of(bi):
                if bi == 0:
                    return a_h0[:]
                if bi == 1:
                    return a_h1[:]
                if bi < 4:
                    return a_h23[:, bi - 2, :]
                if bi < NBF:
                    return a_h45[:, bi - 4, :]
                if bi < 8:
                    return a_mid[:, bi - NBF, :]
                return a_hi[:, bi - 8, :]

            def b_of(bi):
                if bi == 0:
                    return b_h0[:]
                if bi == 1:
                    return b_h1[:]
                if bi < 4:
                    return b_h23[:, bi - 2, :]
                if bi < NBF:
                    return b_h45[:, bi - 4, :]
                if bi < 8:
                    return b_mid[:, bi - NBF, :]
                return b_hi[:, bi - 8, :]

            # PE warm-up: ~3.4us of dummy matmuls on a zeroed tile so the
            # HAM clock-gate un-throttles before the first real matmul lands.
            wrm = ipool.tile([128, 640], bf16, tag="wrm")
            nc.vector.memset(wrm[:], 0.0)
            # ACT warm-up: trigger the one-time ~2.7us ACT_TABLE_LOAD for the
            # Copy function set now, while inputs are still loading, so the
            # first real drain doesn't pay it
            scr = ipool.tile([128, 8], i8, tag="scr")
            nc.scalar.activation(
                out=scr[:],
                in_=wrm[:, 0:8],
                func=mybir.ActivationFunctionType.Copy,
                scale=1.0,
            )
            ps0 = pspool.tile([128, 2, 512], f32, tag="ps")
            for w in range(8):
                nc.tensor.matmul(
                    ps0[:, w % 2, :],
                    wrm[:, 0:128],
                    wrm[:, 128:640],
                    start=True,
                    stop=True,
                )

            # Batches 0..NBF-1: host-prepared bf16 via the two HWDGE rings in
            # parallel, one DMA per batch so batch 0 lands ASAP (no cast, no
            # engine work). Batches NBF-15: SWDGE cast-DMA (int8->bf16 in the
            # DMA datapath) in 4 big chunks to amortize the ~2.7us per-DMA Q7
            # issue cost.
            # prologue loads: exactly 2 DMAs per HWDGE ring. A 4th DMA on a
            # ring recycles a DMA sem lane and its issue WAITS inside the
            # sequencer queue; extra issues on sync also delay the first
            # stores (y_sb recycling). a-batches 0-1 land first (~12us), the
            # rest by ~18us, well before they are consumed.
            a_bv = a_bf.rearrange("b k m -> k b m")
            b_bv = b_bf.rearrange("b k n -> k b n")
            nc.sync.dma_start(out=a_h0[:], in_=a_bf[0])
            nc.scalar.dma_start(out=b_h0[:], in_=b_bf[0])
            nc.sync.dma_start(out=a_h1[:], in_=a_bf[1])
            nc.scalar.dma_start(out=b_h1[:], in_=b_bf[1])
            nc.sync.dma_start(out=a_h23[:], in_=a_bv[:, 2:4, :])
            nc.scalar.dma_start(out=b_h23[:], in_=b_bv[:, 2:4, :])
            # 4th/5th DMA on a ring recycles a sem lane and its issue waits in
            # the sequencer queue - keep those on sync (harmless) and keep the
            # scalar (ACT) ring at 3 issues
            nc.sync.dma_start(out=a_h45[:], in_=a_bv[:, 4:NBF, :])
            nc.sync.dma_start(out=b_h45[:], in_=b_bv[:, 4:NBF, :])
            nc.gpsimd.dma_start(out=a_mid[:], in_=a_v[:, NBF:8, :])
            nc.gpsimd.dma_start(out=b_mid[:], in_=b_v[:, NBF:8, :])
            nc.gpsimd.dma_start(out=a_hi[:], in_=a_v[:, 8:16, :])
            nc.gpsimd.dma_start(out=b_hi[:], in_=b_v[:, 8:16, :])

            di = 0  # drain-op index for ACT/DVE balancing
            for bi in range(BPC):
                at = a_of(bi)  # [128, 1024] k x m
                bt = b_of(bi)  # [128, 1024] k x n
                y_sb = opool.tile([128, MT, N], i8, tag="y")

                for mt in range(MT):  # one 2-bank psum tile per m-tile
                    ps = pspool.tile([128, 2, 512], f32, tag="ps")
                    for nh in range(2):
                        nc.tensor.matmul(
                            ps[:, nh, :],
                            at[:, mt * 128 : (mt + 1) * 128],
                            bt[:, nh * 512 : (nh + 1) * 512],
                            start=True,
                            stop=True,
                        )
                    # the very last tile drains as two 512-el halves on both
                    # engines in parallel: ends ~450ns sooner and rebalances
                    # the final half-op onto the less-loaded DVE
                    if di == 127:
                        nc.scalar.activation(
                            out=y_sb[:, mt, 0:512],
                            in_=ps[:, 0, :],
                            func=mybir.ActivationFunctionType.Copy,
                            scale=float(alpha),
                        )
                        nc.vector.tensor_scalar(
                            out=y_sb[:, mt, 512:1024],
                            in0=ps[:, 1, :],
                            scalar1=float(alpha),
                            scalar2=None,
                            op0=mybir.AluOpType.mult,
                        )
                        di += 1
                        yv7 = y[bi].rearrange("(t p) n -> p t n", p=128)
                        nc.sync.dma_start(
                            out=yv7[:, 7, 0:512], in_=y_sb[:, 7, 0:512]
                        )
                        nc.sync.dma_start(
                            out=yv7[:, 7, 512:1024], in_=y_sb[:, 7, 512:1024]
                        )
                        continue
                    dst = y_sb[:, mt, :]
                    src = ps.rearrange("p a b -> p (a b)")
                    # strict A/D interleave with 3 extra ACT tiles: 67 ACT vs
                    # 61 DVE balances measured 1122ns vs 1215ns per-op costs;
                    # di=127 in the flip set puts the last drain on the
                    # faster ACT engine so the final store goes out sooner
                    if di % 2 == 0 or di in (21, 63, 127):
                        nc.scalar.activation(
                            out=dst,
                            in_=src,
                            func=mybir.ActivationFunctionType.Copy,
                            scale=float(alpha),
                        )
                    else:
                        nc.vector.tensor_scalar(
                            out=dst,
                            in0=src,
                            scalar1=float(alpha),
                            scalar2=None,
                            op0=mybir.AluOpType.mult,
                        )
                    di += 1
                    # half-batch stores (512 KiB) as soon as each half is
                    # drained, on the otherwise-idle sync ring only. NEVER on
                    # gpsimd: SWDGE descriptor generation thrashes SBUF ports
                    # and slows every concurrent ACT/DVE op ~20%. The last
                    # batch stores ever-finer so the final store after the
                    # last drain is only 128 KiB; mt6's store issues from the
                    # scalar ring in ACT's post-drain shadow.
                    yv = y[bi].rearrange("(t p) n -> p t n", p=128)
                    last = bi == BPC - 1
                    if mt == 3:
                        nc.sync.dma_start(out=yv[:, 0:4, :], in_=y_sb[:, 0:4, :])
                    elif not last and mt == 7:
                        nc.sync.dma_start(out=yv[:, 4:8, :], in_=y_sb[:, 4:8, :])
                    elif last and mt == 5:
                        nc.sync.dma_start(out=yv[:, 4:6, :], in_=y_sb[:, 4:6, :])
                    elif last and mt == 6:
                        nc.scalar.dma_start(out=yv[:, 6, :], in_=y_sb[:, 6, :])
                    elif last and mt == 7:
                        nc.sync.dma_start(out=yv[:, 7, :], in_=y_sb[:, 7, :])

    nc.compile()
    return nc


def _get_nc(alpha: float):
    key = float(alpha)
    if key not in _cache:
        _cache[key] = _build(key)
    return _cache[key]


def _shard_inputs(a, b):
    import ml_dtypes

    # host-side pre-transpose to [B, K, M] / [B, K, N]
    a_t = np.ascontiguousarray(a.transpose(0, 2, 1))
    b_t = np.ascontiguousarray(b.transpose(0, 2, 1))
    maps = []
    for c in range(N_CORES):
        at = a_t[c * BPC : (c + 1) * BPC]
        bt = b_t[c * BPC : (c + 1) * BPC]
        maps.append(
            {
                "a_t": at,
                "b_t": bt,
                "a_bf": at[:NBF].astype(ml_dtypes.bfloat16),
                "b_bf": bt[:NBF].astype(ml_dtypes.bfloat16),
            }
        )
    return maps


def kernel(a, b, alpha):
    from concourse.bass_utils import run_bass_kernel_spmd

    a = np.asarray(a)
    b = np.asarray(b)
    assert a.shape == (B, M, K) and a.dtype == np.int8
    assert b.shape == (B, N, K) and b.dtype == np.int8

    nc = _get_nc(float(alpha))
    in_maps = _shard_inputs(a, b)
    res = run_bass_kernel_spmd(nc, in_maps, list(range(N_CORES)))
    out = np.concatenate([r["y"] for r in res.results], axis=0)
    return out.astype(np.int8)
